# revision 6
# baseline (speedup 1.0000x reference)
"""Trainium2 Bass kernel: separable parabolic morphological dilation (11-tap).

nn_Dilation2dSingle: im [8, 32, 512, 512] f32, se_coef scalar, se [11, 1].
    bias[k] = se_coef * se[k, 0]           (parabolic, symmetric, bias[5] = 0)
    out = vdilate(hdilate(im)) with NEG=-10000 padding.

Per 1D pass the parabolic window decomposes into symmetric pairs:
    y[i] = max(x[i], max_{d=1..5}( max(x[i-d], x[i+d]) - b_d ))
which is 5 tensor_max + 5 fused scalar_tensor_tensor (subtract+max) DVE ops
per data pass -- DVE is the bottleneck engine; the vertical axis is handled
by PE identity-matmul transposes with ACT doing PSUM->SBUF copies, so PE/ACT/
DMA all hide under DVE. fp32 throughout: bitwise-exact vs the reference.

Sharding: pure data-parallel over batch (8 cores x 1 batch each), se-derived
bias and a 128x128 identity are tiny replicated inputs; no collectives.
"""

from contextlib import ExitStack

import numpy as np

import concourse.bacc as bacc
import concourse.mybir as mybir
import concourse.tile as tile
from concourse.bass_utils import run_bass_kernel_spmd

F32 = mybir.dt.float32
NEG = -10000.0
R = 5  # dilation radius (window 11)

# Hardcoded problem shape (per spec).
B, C, H, W = 8, 32, 512, 512
N_CORES = 8
CP = 4  # channels packed per DVE instruction group


def _dilate_free(nc, pd_pool, acc_view, src3, bias_tile, n_seg, L):
    """1D dilation along the innermost free axis.

    src3: AP [128, n_seg, L + 2R] (NEG-padded segments)
    acc_view: AP [128, n_seg, L] output
    bias_tile: SBUF [128, R]; column d-1 holds b_d replicated over partitions
    """
    center = src3[:, :, R : R + L]
    for d in range(1, R + 1):
        pd = pd_pool.tile([128, n_seg * L], F32, tag="pd")
        pdv = pd[:].rearrange("p (s c) -> p s c", s=n_seg)
        nc.vector.tensor_max(
            pdv, src3[:, :, R - d : R - d + L], src3[:, :, R + d : R + d + L]
        )
        prev = center if d == 1 else acc_view
        nc.vector.scalar_tensor_tensor(
            acc_view,
            pdv,
            bias_tile[:, d - 1 : d],
            prev,
            op0=mybir.AluOpType.subtract,
            op1=mybir.AluOpType.max,
        )


def build_nc(C=C, H=H, W=W, CP=CP, reps=1):
    """Build the per-core Bass module.

    reps > 1 repeats the whole pipeline (same output) -- used only by the
    test harness for differential device-time measurement."""
    assert H % 128 == 0 and W % 128 == 0 and C % CP == 0
    nH, nW, nG = H // 128, W // 128, C // CP
    SW, SH = W + 2 * R, H + 2 * R

    nc = bacc.Bacc("TRN2", target_bir_lowering=False, debug=False)
    im = nc.dram_tensor("im", [C, H, W], F32, kind="ExternalInput")
    bias = nc.dram_tensor("bias5", [128, R], F32, kind="ExternalInput")
    iden = nc.dram_tensor("iden", [128, 128], F32, kind="ExternalInput")
    out = nc.dram_tensor("out", [C, H, W], F32, kind="ExternalOutput")

    with tile.TileContext(nc) as tc, ExitStack() as ctx:
        const_pool = ctx.enter_context(tc.tile_pool(name="const", bufs=1))
        hin_pool = ctx.enter_context(tc.tile_pool(name="hin", bufs=3))
        pd_pool = ctx.enter_context(tc.tile_pool(name="pd", bufs=3))
        hacc_pool = ctx.enter_context(tc.tile_pool(name="hacc", bufs=nH + 2))
        vin_pool = ctx.enter_context(tc.tile_pool(name="vin", bufs=3))
        vacc_pool = ctx.enter_context(tc.tile_pool(name="vacc", bufs=nW + 2))
        st_pool = ctx.enter_context(tc.tile_pool(name="st", bufs=6))
        psf_pool = ctx.enter_context(tc.tile_pool(name="psf", bufs=3, space="PSUM"))
        psb_pool = ctx.enter_context(tc.tile_pool(name="psb", bufs=3, space="PSUM"))

        identity = const_pool.tile([128, 128], F32)
        nc.sync.dma_start(identity[:], iden.ap())
        bias_t = const_pool.tile([128, R], F32)
        nc.sync.dma_start(bias_t[:], bias.ap())

        for _rep in range(reps):
          for g in range(nG):
            # ---- horizontal pass over nH row-tiles ----
            haccs = []
            for t in range(nH):
                ht = hin_pool.tile([128, CP * SW], F32, tag="hin")
                for ci in range(CP):
                    b0 = ci * SW
                    nc.gpsimd.memset(ht[:, b0 : b0 + R], NEG)
                    nc.gpsimd.memset(ht[:, b0 + R + W : b0 + SW], NEG)
                    nc.sync.dma_start(
                        ht[:, b0 + R : b0 + R + W],
                        im.ap()[g * CP + ci, t * 128 : (t + 1) * 128, :],
                    )
                acc = hacc_pool.tile([128, CP * W], F32, tag="hacc")
                accv = acc[:].rearrange("p (s c) -> p s c", s=CP)
                src3 = ht[:].rearrange("p (s c) -> p s c", s=CP)
                _dilate_free(nc, pd_pool, accv, src3, bias_t, CP, W)
                haccs.append(acc)

            # ---- transpose + vertical pass over nW col-tiles ----
            vaccs = []
            for w in range(nW):
                vt = vin_pool.tile([128, CP * SH], F32, tag="vin")
                for ci in range(CP):
                    b0 = ci * SH
                    nc.gpsimd.memset(vt[:, b0 : b0 + R], NEG)
                    nc.gpsimd.memset(vt[:, b0 + R + H : b0 + SH], NEG)
                    pt = psf_pool.tile([128, H], F32, tag="psf")
                    for t in range(nH):
                        nc.tensor.transpose(
                            pt[:, t * 128 : (t + 1) * 128],
                            haccs[t][:, ci * W + w * 128 : ci * W + (w + 1) * 128],
                            identity[:],
                        )
                    nc.scalar.copy(vt[:, b0 + R : b0 + R + H], pt[:])
                vacc = vacc_pool.tile([128, CP * H], F32, tag="vacc")
                vaccv = vacc[:].rearrange("p (s c) -> p s c", s=CP)
                vsrc3 = vt[:].rearrange("p (s c) -> p s c", s=CP)
                _dilate_free(nc, pd_pool, vaccv, vsrc3, bias_t, CP, H)
                vaccs.append(vacc)

            # ---- transpose back + store ----
            for ci in range(CP):
                for t in range(nH):
                    qt = psb_pool.tile([128, W], F32, tag="psb")
                    for w in range(nW):
                        nc.tensor.transpose(
                            qt[:, w * 128 : (w + 1) * 128],
                            vaccs[w][:, ci * H + t * 128 : ci * H + (t + 1) * 128],
                            identity[:],
                        )
                    st = st_pool.tile([128, W], F32, tag="st")
                    nc.scalar.copy(st[:], qt[:])
                    nc.sync.dma_start(
                        out.ap()[g * CP + ci, t * 128 : (t + 1) * 128, :], st[:]
                    )

    nc.compile()
    return nc


_NC_CACHE = {}


def _get_nc():
    if "nc" not in _NC_CACHE:
        _NC_CACHE["nc"] = build_nc()
    return _NC_CACHE["nc"]


def _make_in_maps(im, se_coef, se):
    im = np.ascontiguousarray(np.asarray(im, dtype=np.float32))
    se = np.asarray(se, dtype=np.float32)
    se_coef = np.asarray(se_coef, dtype=np.float32)
    bias11 = (se_coef * se[:, 0]).astype(np.float32)  # same fp32 op as reference
    bias5 = np.ascontiguousarray(
        np.broadcast_to(bias11[R + 1 : 2 * R + 1], (128, R))
    ).astype(np.float32)
    iden = np.eye(128, dtype=np.float32)
    return [
        {"im": im[b], "bias5": bias5, "iden": iden} for b in range(im.shape[0])
    ]


def kernel(im, se_coef, se):
    nc = _get_nc()
    in_maps = _make_in_maps(im, se_coef, se)
    res = run_bass_kernel_spmd(nc, in_maps, core_ids=list(range(N_CORES)))
    out = np.stack([res.results[b]["out"] for b in range(N_CORES)], axis=0)
    return out.astype(np.float32)


# revision 9
# speedup vs baseline: 1.0004x; 1.0004x over previous
"""Trainium2 Bass kernel: separable parabolic morphological dilation (11-tap).

nn_Dilation2dSingle: im [8, 32, 512, 512] f32, se_coef scalar, se [11, 1].
    bias[k] = se_coef * se[k, 0]           (parabolic, symmetric, bias[5] = 0)
    out = vdilate(hdilate(im)) with NEG=-10000 padding.

Per 1D pass the parabolic window decomposes into symmetric pairs:
    y[i] = max(x[i], max_{d=1..5}( max(x[i-d], x[i+d]) - b_d ))
which is 5 tensor_max + 5 fused scalar_tensor_tensor (subtract+max) DVE ops
per data pass -- DVE is the bottleneck engine; the vertical axis is handled
by PE identity-matmul transposes with ACT doing PSUM->SBUF copies, so PE/ACT/
DMA all hide under DVE. fp32 throughout: bitwise-exact vs the reference.

Sharding: pure data-parallel over batch (8 cores x 1 batch each), se-derived
bias and a 128x128 identity are tiny replicated inputs; no collectives.
"""

from contextlib import ExitStack

import numpy as np

import concourse.bacc as bacc
import concourse.mybir as mybir
import concourse.tile as tile
from concourse.bass_utils import run_bass_kernel_spmd

F32 = mybir.dt.float32
NEG = -10000.0
R = 5  # dilation radius (window 11)

# Hardcoded problem shape (per spec).
B, C, H, W = 8, 32, 512, 512
N_CORES = 8
CP = 4  # channels packed per DVE instruction group


def _dilate_free(nc, pd_pool, acc_view, src3, bias_tile, n_seg, L):
    """1D dilation along the innermost free axis.

    src3: AP [128, n_seg, L + 2R] (NEG-padded segments)
    acc_view: AP [128, n_seg, L] output
    bias_tile: SBUF [128, R]; column d-1 holds b_d replicated over partitions
    """
    center = src3[:, :, R : R + L]
    for d in range(1, R + 1):
        pd = pd_pool.tile([128, n_seg * L], F32, tag="pd")
        pdv = pd[:].rearrange("p (s c) -> p s c", s=n_seg)
        nc.vector.tensor_max(
            pdv, src3[:, :, R - d : R - d + L], src3[:, :, R + d : R + d + L]
        )
        prev = center if d == 1 else acc_view
        nc.vector.scalar_tensor_tensor(
            acc_view,
            pdv,
            bias_tile[:, d - 1 : d],
            prev,
            op0=mybir.AluOpType.subtract,
            op1=mybir.AluOpType.max,
        )


def build_nc(C=C, H=H, W=W, CP=CP, reps=1):
    """Build the per-core Bass module.

    reps > 1 repeats the whole pipeline (same output) -- used only by the
    test harness for differential device-time measurement."""
    assert H % 128 == 0 and W % 128 == 0 and C % CP == 0
    nH, nW, nG = H // 128, W // 128, C // CP
    SW, SH = W + 2 * R, H + 2 * R

    nc = bacc.Bacc("TRN2", target_bir_lowering=False, debug=False)
    im = nc.dram_tensor("im", [C, H, W], F32, kind="ExternalInput")
    bias = nc.dram_tensor("bias5", [128, R], F32, kind="ExternalInput")
    iden = nc.dram_tensor("iden", [128, 128], F32, kind="ExternalInput")
    out = nc.dram_tensor("out", [C, H, W], F32, kind="ExternalOutput")

    with tile.TileContext(nc) as tc, ExitStack() as ctx:
        const_pool = ctx.enter_context(tc.tile_pool(name="const", bufs=1))
        hin_pool = ctx.enter_context(tc.tile_pool(name="hin", bufs=3))
        pd_pool = ctx.enter_context(tc.tile_pool(name="pd", bufs=3))
        hacc_pool = ctx.enter_context(tc.tile_pool(name="hacc", bufs=nH + 2))
        vin_pool = ctx.enter_context(tc.tile_pool(name="vin", bufs=3))
        vacc_pool = ctx.enter_context(tc.tile_pool(name="vacc", bufs=nW + 2))
        st_pool = ctx.enter_context(tc.tile_pool(name="st", bufs=6))
        psf_pool = ctx.enter_context(tc.tile_pool(name="psf", bufs=3, space="PSUM"))
        psb_pool = ctx.enter_context(tc.tile_pool(name="psb", bufs=3, space="PSUM"))

        identity = const_pool.tile([128, 128], F32)
        nc.sync.dma_start(identity[:], iden.ap())
        bias_t = const_pool.tile([128, R], F32)
        nc.sync.dma_start(bias_t[:], bias.ap())
        # Constant NEG source for halo pads. Pads are written by ACT copies
        # (not gpsimd memsets): GPSIMD shares the DVE's SBUF port, so Q7
        # launches in the hot loop would steal cycles from the DVE stream.
        neg_t = const_pool.tile([128, CP * R], F32)
        nc.gpsimd.memset(neg_t[:], NEG)

        def set_pads(tile_, seg):
            """Write NEG into the 2*CP halo pad blocks with 2 strided copies."""
            v = tile_[:].rearrange("p (s c) -> p s c", s=CP)
            src = neg_t[:].rearrange("p (s c) -> p s c", s=CP)
            nc.scalar.copy(v[:, :, 0:R], src)
            nc.scalar.copy(v[:, :, seg - R : seg], src)

        for _rep in range(reps):
          for g in range(nG):
            # ---- horizontal pass over nH row-tiles ----
            haccs = []
            for t in range(nH):
                ht = hin_pool.tile([128, CP * SW], F32, tag="hin")
                set_pads(ht, SW)
                for ci in range(CP):
                    b0 = ci * SW
                    nc.sync.dma_start(
                        ht[:, b0 + R : b0 + R + W],
                        im.ap()[g * CP + ci, t * 128 : (t + 1) * 128, :],
                    )
                acc = hacc_pool.tile([128, CP * W], F32, tag="hacc")
                accv = acc[:].rearrange("p (s c) -> p s c", s=CP)
                src3 = ht[:].rearrange("p (s c) -> p s c", s=CP)
                _dilate_free(nc, pd_pool, accv, src3, bias_t, CP, W)
                haccs.append(acc)

            # ---- transpose + vertical pass over nW col-tiles ----
            vaccs = []
            for w in range(nW):
                vt = vin_pool.tile([128, CP * SH], F32, tag="vin")
                set_pads(vt, SH)
                for ci in range(CP):
                    b0 = ci * SH
                    pt = psf_pool.tile([128, H], F32, tag="psf")
                    for t in range(nH):
                        nc.tensor.transpose(
                            pt[:, t * 128 : (t + 1) * 128],
                            haccs[t][:, ci * W + w * 128 : ci * W + (w + 1) * 128],
                            identity[:],
                        )
                    nc.scalar.copy(vt[:, b0 + R : b0 + R + H], pt[:])
                vacc = vacc_pool.tile([128, CP * H], F32, tag="vacc")
                vaccv = vacc[:].rearrange("p (s c) -> p s c", s=CP)
                vsrc3 = vt[:].rearrange("p (s c) -> p s c", s=CP)
                _dilate_free(nc, pd_pool, vaccv, vsrc3, bias_t, CP, H)
                vaccs.append(vacc)

            # ---- transpose back + store ----
            for ci in range(CP):
                for t in range(nH):
                    qt = psb_pool.tile([128, W], F32, tag="psb")
                    for w in range(nW):
                        nc.tensor.transpose(
                            qt[:, w * 128 : (w + 1) * 128],
                            vaccs[w][:, ci * H + t * 128 : ci * H + (t + 1) * 128],
                            identity[:],
                        )
                    st = st_pool.tile([128, W], F32, tag="st")
                    nc.scalar.copy(st[:], qt[:])
                    nc.sync.dma_start(
                        out.ap()[g * CP + ci, t * 128 : (t + 1) * 128, :], st[:]
                    )

    nc.compile()
    return nc


_NC_CACHE = {}


def _get_nc():
    if "nc" not in _NC_CACHE:
        _NC_CACHE["nc"] = build_nc()
    return _NC_CACHE["nc"]


def _make_in_maps(im, se_coef, se):
    im = np.ascontiguousarray(np.asarray(im, dtype=np.float32))
    se = np.asarray(se, dtype=np.float32)
    se_coef = np.asarray(se_coef, dtype=np.float32)
    bias11 = (se_coef * se[:, 0]).astype(np.float32)  # same fp32 op as reference
    bias5 = np.ascontiguousarray(
        np.broadcast_to(bias11[R + 1 : 2 * R + 1], (128, R))
    ).astype(np.float32)
    iden = np.eye(128, dtype=np.float32)
    return [
        {"im": im[b], "bias5": bias5, "iden": iden} for b in range(im.shape[0])
    ]


def kernel(im, se_coef, se):
    nc = _get_nc()
    in_maps = _make_in_maps(im, se_coef, se)
    res = run_bass_kernel_spmd(nc, in_maps, core_ids=list(range(N_CORES)))
    out = np.stack([res.results[b]["out"] for b in range(N_CORES)], axis=0)
    return out.astype(np.float32)
